# revision 3
# baseline (speedup 1.0000x reference)
"""Trainium2 Bass kernel for nn_L2PppMaskAttn (topk_masking) — v5.

v6 + stall/clock fixes (see git of v5/v6 for lineage):
  - K/A DMA issues precede the activation-table warm on the scalar
    stream; P ships from the host as bf16 (half the P HBM traffic).
  - xp bufs=4, ps_c bufs=3: the PE was stalling on the x-tile and
    score-psum rings every few iterations.
  - PSUM->SBUF copies split scalar 9 / vector 3.
  - Last four groups' writes are chunked and spread across all three
    DMA queues to kill the end-of-kernel drain tail.
Previous (v5/v6) notes:
  - x loads on the gpsimd (SWDGE) queue so they never sit behind 3MB
    output writes on the HWDGE queues.
  - P-pool slot0 load gated behind a dummy gpsimd op that depends on
    the K load: gpsimd would otherwise fire the 2.4MB P DMA at t=0 and
    starve the critical K/A loads on the shared SDMA engines.
  - mask-transpose + wt-scale hoisted one iteration ahead (w_phase) so
    the PE's out-matmuls never wait on the current group's topk.
  - Final two groups write in 3 chunks to shrink the end-of-kernel
    write-drain tail.
"""

import sys

sys.path.insert(0, "/opt/trn_rl_repo")

import numpy as np

B, L, P_N, LP, D = 1024, 12, 100, 8, 768
N_CORES = 8
NF = LP * D
TOP_K = 5
NEG_BIG = -1.0e30
ROWS = B + B // 2
N_SLOTS = 2
N_GROUPS = ROWS // 128  # 12

_CACHE = {}


def _build_nc():
    if "nc" in _CACHE:
        return _CACHE["nc"]

    from contextlib import ExitStack

    import concourse.bass as bass
    import concourse.bacc as bacc
    import concourse.mybir as mybir
    from concourse import masks
    from concourse.tile import TileContext

    f32 = mybir.dt.float32
    bf16 = mybir.dt.bfloat16
    AX = mybir.AxisListType
    OP = mybir.AluOpType
    AF = mybir.ActivationFunctionType

    nc = bacc.Bacc(
        "TRN2",
        target_bir_lowering=False,
        debug=False,
        num_devices=N_CORES,
    )

    # x is the HOST-TRANSPOSED layout: row r = (group r//128, d_lo r%128),
    # free dim = j*128+b (j = d-block, b = batch row within group).
    x_d = nc.declare_dram_parameter("x", [ROWS, D], f32, isOutput=False)
    k_d = nc.declare_dram_parameter("k", [N_SLOTS, P_N, D], f32, isOutput=False)
    a_d = nc.declare_dram_parameter("a", [N_SLOTS, P_N, D], f32, isOutput=False)
    p_d = nc.declare_dram_parameter("p", [N_SLOTS, P_N, NF], bf16, isOutput=False)
    o_d = nc.declare_dram_parameter("o", [ROWS, NF], f32, isOutput=True)

    with TileContext(nc) as tc, ExitStack() as ctx:
        pool = lambda name, bufs, **kw: ctx.enter_context(
            tc.tile_pool(name=name, bufs=bufs, **kw)
        )
        const = pool("const", 1)
        xp = pool("xp", 4)
        kap = pool("kap", 2)
        scrp = pool("scrp", 2)
        nrm = pool("nrm", 2)
        nktp = pool("nktp", 2)
        ppool = pool("pp", 2)
        obuf = pool("ob", 3)
        small = pool("small", 3)
        rowp = pool("rowp", 2)
        wtp = pool("wtp", 2)
        ps_t = pool("ps_t", 2, space="PSUM")
        ps_c = pool("ps_c", 2, space="PSUM")
        ps_o = pool("ps_o", 4, space="PSUM")

        ident = const.tile([128, 128], f32)
        masks.make_identity(nc, ident[:])

        def grow(i):
            return i * 128

        def gslot(i):
            return 0 if i < 8 else 1

        x_tiles = [None] * N_GROUPS
        pc_tiles = [None] * N_GROUPS
        mask_tiles = [None] * N_GROUPS
        slot_ctx = [None] * N_SLOTS

        def xload(i):
            t = xp.tile([128, D], f32)
            nc.gpsimd.dma_start(t[:], x_d[grow(i) : grow(i) + 128])
            x_tiles[i] = t

        def load_ka(slot):
            ka = kap.tile([P_N, D], f32, tag="ka")
            nc.sync.dma_start(ka[:], k_d[slot])
            aa = kap.tile([P_N, D], f32, tag="aa")
            nc.sync.dma_start(aa[:], a_d[slot])
            return ka, aa

        def load_p(slot):
            p_sb = ppool.tile([P_N, NF], bf16)
            nc.gpsimd.dma_start(p_sb[:], p_d[slot])
            return p_sb

        def prep_norms(slot, ka, aa):
            rs = []
            for src in (ka, aa):
                scr = scrp.tile([P_N, D], f32, tag="scr")
                nc.vector.tensor_tensor(scr[:], src[:], src[:], op=OP.mult)
                ss = small.tile([P_N, 1], f32, tag="ss")
                nc.vector.reduce_sum(ss[:], scr[:], axis=AX.X)
                sq = small.tile([P_N, 1], f32, tag="sq")
                nc.scalar.activation(sq[:], ss[:], AF.Sqrt)
                y0 = small.tile([P_N, 1], f32, tag="y0")
                nc.vector.reciprocal(y0[:], sq[:])
                t1 = small.tile([P_N, 1], f32, tag="t1")
                nc.vector.tensor_tensor(t1[:], y0[:], y0[:], op=OP.mult)
                nc.vector.tensor_tensor(t1[:], t1[:], ss[:], op=OP.mult)
                nc.vector.tensor_scalar(t1[:], t1[:], -0.5, 1.5, OP.mult, OP.add)
                y1 = small.tile([P_N, 1], f32, tag="y1")
                nc.vector.tensor_tensor(y1[:], t1[:], y0[:], op=OP.mult)
                rs.append(y1)

            nk = nrm.tile([P_N, D], f32, tag="nk")
            nc.vector.tensor_scalar_mul(nk[:], ka[:], rs[0][:])
            na = nrm.tile([P_N, D], f32, tag="na")
            nc.vector.tensor_scalar_mul(na[:], aa[:], rs[1][:])

            scr2 = scrp.tile([P_N, D], f32, tag="scr")
            s_t = small.tile([P_N, 1], f32, tag="s_t")
            nc.vector.tensor_tensor(scr2[:], nk[:], na[:], op=OP.mult)
            nc.vector.reduce_sum(s_t[:], scr2[:], axis=AX.X)

            nkt = nktp.tile([128, 6 * P_N], f32)
            for j in range(6):
                pt = ps_t.tile([128, P_N], f32, tag="tp")
                nc.tensor.transpose(
                    pt[:], nk[:, j * 128 : (j + 1) * 128], ident[:P_N, :P_N]
                )
                nc.scalar.copy(nkt[:, j * P_N : (j + 1) * P_N], pt[:])
            return nkt, s_t

        def s_phase(i):
            nkt, _, _ = slot_ctx[gslot(i)]
            xt = x_tiles[i]  # already transposed by the host
            pc = ps_c.tile([128, P_N], f32)
            for j in range(6):
                nc.tensor.matmul(
                    pc[:],
                    xt[:, j * 128 : (j + 1) * 128],
                    nkt[:, j * P_N : (j + 1) * P_N],
                    start=(j == 0),
                    stop=(j == 5),
                )
            pc_tiles[i] = pc

        def t_phase(i):
            pc = pc_tiles[i]
            work = rowp.tile([128, P_N], f32, tag="work")
            mm = small.tile([128, TOP_K], f32, tag="mm")
            pen = rowp.tile([128, P_N], f32, tag="pen")
            for it in range(TOP_K):
                src = pc if it == 0 else work
                nc.vector.reduce_max(mm[:, it : it + 1], src[:], axis=AX.X)
                if it < TOP_K - 1:
                    nc.vector.tensor_scalar(
                        pen[:], src[:], mm[:, it : it + 1], NEG_BIG, OP.is_ge, OP.mult
                    )
                    nc.vector.tensor_tensor(work[:], src[:], pen[:], op=OP.add)
            mask = rowp.tile([128, P_N], f32, tag="mask")
            nc.vector.tensor_scalar(
                mask[:], pc[:], mm[:, TOP_K - 1 : TOP_K], None, OP.is_ge
            )
            mask_tiles[i] = mask

        wt_tiles = [None] * N_GROUPS

        def w_phase(i):
            # mask transpose (PE) + wt scale (vector), one iteration ahead
            _, s_t, _ = slot_ctx[gslot(i)]
            mask = mask_tiles[i]
            mt = ps_t.tile([P_N, 128], f32, tag="tp")
            nc.tensor.transpose(mt[:], mask[:], ident[:])
            wt = wtp.tile([P_N, 128], bf16)
            nc.vector.tensor_scalar_mul(wt[:], mt[:], s_t[:])
            wt_tiles[i] = wt

        def o_phase(i):
            _, s_t, p_sb = slot_ctx[gslot(i)]
            wt = wt_tiles[i]
            ob = obuf.tile([128, NF], f32)
            chunked = i >= N_GROUPS - 4
            chunk_engs = [nc.sync, nc.scalar, nc.gpsimd]
            for n in range(12):
                po = ps_o.tile([128, 512], f32)
                nc.tensor.matmul(
                    po[:], wt[:], p_sb[:, n * 512 : (n + 1) * 512], start=True, stop=True
                )
                if n in (1, 3, 6, 8, 10):
                    nc.vector.tensor_copy(ob[:, n * 512 : (n + 1) * 512], po[:])
                else:
                    nc.scalar.copy(ob[:, n * 512 : (n + 1) * 512], po[:])
                if chunked and n % 4 == 3:
                    c0 = (n - 3) * 512
                    eng = chunk_engs[(i + n // 4) % 3]
                    eng.dma_start(
                        o_d[grow(i) : grow(i) + 128, c0 : c0 + 2048],
                        ob[:, c0 : c0 + 2048],
                    )
            if not chunked:
                nc.sync.dma_start(o_d[grow(i) : grow(i) + 128], ob[:])

        # ---- prologue ----
        ka0, aa0 = load_ka(0)
        # warm the Sqrt table while K/A fly
        warm = const.tile([1, 1], f32)
        nc.scalar.activation(warm[:], ident[:1, :1], AF.Sqrt)
        # gate: gpsimd op depending on the K load, so x/P DMAs on the
        # gpsimd queue fire only after the critical K/A rows land
        gate = const.tile([1, 1], f32)
        nc.gpsimd.tensor_copy(gate[:], ka0[:1, :1])
        xload(0)
        xload(1)
        p0 = load_p(0)
        nkt0, st0 = prep_norms(0, ka0, aa0)
        slot_ctx[0] = (nkt0, st0, p0)
        s_phase(0)
        t_phase(0)
        w_phase(0)

        # ---- pipelined main loop ----
        prep1 = [None]
        for i in range(N_GROUPS):
            if i + 2 < N_GROUPS:
                xload(i + 2)
            if i == 1:
                ka1, aa1 = load_ka(1)
                prep1[0] = (ka1, aa1, load_p(1))
            if i == 3:
                ka1, aa1, p1 = prep1[0]
                nkt1, st1 = prep_norms(1, ka1, aa1)
                slot_ctx[1] = (nkt1, st1, p1)
            if i + 1 < N_GROUPS:
                s_phase(i + 1)
                t_phase(i + 1)
            o_phase(i)
            if i + 1 < N_GROUPS:
                w_phase(i + 1)

    nc.compile()
    _CACHE["nc"] = nc
    return nc


def _run(x_query, K_all, A_all, P_all, trace=False, tmpdir=None):
    from concourse.bass_utils import run_bass_kernel_spmd

    x = np.ascontiguousarray(np.asarray(x_query, dtype=np.float32))
    k = np.asarray(K_all, dtype=np.float32)
    a = np.asarray(A_all, dtype=np.float32)
    import ml_dtypes

    p = np.asarray(P_all, dtype=np.float32).reshape(L, P_N, NF).astype(
        ml_dtypes.bfloat16
    )

    nc = _build_nc()
    in_maps = []
    for c in range(N_CORES):
        l2 = 8 + c // 2
        h = c % 2
        x_pack = np.concatenate(
            [x[:, c, :], x[h * 512 : (h + 1) * 512, l2, :]], axis=0
        )  # [1536, 768]
        # transpose to the block layout the score matmul consumes:
        # xt[g, d_lo, j*128+b] = x_pack[g*128+b, j*128+d_lo]
        xt = (
            x_pack.reshape(N_GROUPS, 128, 6, 128)
            .transpose(0, 3, 2, 1)
            .reshape(ROWS, D)
        )
        idx = [c, l2]
        in_maps.append(
            {
                "x": np.ascontiguousarray(xt),
                "k": np.ascontiguousarray(k[idx]),
                "a": np.ascontiguousarray(a[idx]),
                "p": np.ascontiguousarray(p[idx]),
            }
        )
    br = run_bass_kernel_spmd(
        nc, in_maps, list(range(N_CORES)), trace=trace, tmpdir=tmpdir
    )
    out = np.empty((L, B, LP, D), dtype=np.float32)
    for c in range(N_CORES):
        o = br.results[c]["o"]
        out[c] = o[:B].reshape(B, LP, D)
        l2 = 8 + c // 2
        h = c % 2
        out[l2, h * 512 : (h + 1) * 512] = o[B:].reshape(512, LP, D)
    return out, br


def kernel(x_query, K_all, A_all, P_all):
    out, _ = _run(x_query, K_all, A_all, P_all)
    return out
